# revision 1
# baseline (speedup 1.0000x reference)
"""Trainium2 Bass kernel for nn_AdapterBlock (LN -> dwconv x3 -> SE -> residual).

Data-parallel over batch: 8 samples -> 8 NeuronCores. v3: phase-pipelined,
quarter-granularity convs.

Per core:
  - x loads f32 (plain SWDGE, fast) into a rotating f32 staging buffer;
    the f32->bf16 cast is folded into the LN apply (tensor_scalar)
  - LN stats per t-tile split ACT(accum)/DVE(bn_stats) reading the staging
  - xbar DMA-transpose to layout B per 4-tile group + ACT/DVE re-scatter
  - convs per (channel-group, 512-quarter), interleaved so conv23 trails
    conv1 by one quarter; conv1 windows staggered ([0,511,1023,1535,2047])
    so each window only needs already-scattered tiles; the final column is
    patched by tiny DVE ops
      PE chs: diag-matmul taps accumulated in one PSUM bank per window
      H chs: conv1 on DVE scalar_tensor_tensor chain, conv23 on PE
    SE pool rides the PSUM evacuation (ACT accum_out)
  - c3 output aliases xB (disjoint live ranges)
  - ungated c3 transposes back per channel DURING the conv phase; SE gate
    applied in layout A via a broadcast-row built with a tiny transpose +
    8 PE matmuls; out = stg*gateA + residual per chunk on DVE, SWDGE store
"""

import os
import sys

sys.path.insert(0, "/opt/trn_rl_repo")

from contextlib import ExitStack

import numpy as np

import concourse.bass as bass  # noqa: F401
import concourse.bacc as bacc
import concourse.tile as tile
import concourse.mybir as mybir
from concourse.bass_utils import run_bass_kernel_spmd

B, T, C = 8, 2048, 1024
N_CORES = 8
NT = T // 128          # 16 t-tiles
NCH = C // 128         # 8 channel groups
H = C // 16            # SE hidden = 64
PAD = 4                # zero pad each side of the time axis (>= conv halo 3)
TF = T + 2 * PAD
QT = 512               # conv quarter
EPS = 1e-5

# conv1 windows staggered so window q only reads tiles already scattered
W1 = [0, 511, 1023, 1535, 2047]
W23 = [0, 512, 1024, 1536, 2048]

F32 = mybir.dt.float32
BF16 = mybir.dt.bfloat16
AF = mybir.ActivationFunctionType
OP = mybir.AluOpType

# --- tunables ------------------------------------------------------------
# engine per channel-group: 'P' = both convs on TensorE,
# 'H' = conv1 on DVE + conv23 on TensorE, 'V' = both on DVE
CH_ENG = os.environ.get("K_CH_ENG", "PPPPPPHH")
SCATTER = os.environ.get("K_SCATTER", "split")  # split | dve | act
STATS_ACT = set(int(x) for x in
                os.environ.get("K_STATS_ACT", "0,2,4,6").split(",")
                if x != "")
K_DEBUG = int(os.environ.get("K_DEBUG", "0"))

_CACHE = {}


def _build():
    nc = bacc.Bacc("TRN2", target_bir_lowering=False, debug=False,
                   num_devices=N_CORES)

    x_ext = nc.dram_tensor("x", [T, C], F32, kind="ExternalInput").ap()
    res_ext = nc.dram_tensor("res", [T, C], F32, kind="ExternalInput").ap()
    w1_ext = nc.dram_tensor("w1p", [128, NCH, 3], F32, kind="ExternalInput").ap()
    b1_ext = nc.dram_tensor("b1p", [128, NCH], F32, kind="ExternalInput").ap()
    w23_ext = nc.dram_tensor("w23p", [128, NCH, 7], F32, kind="ExternalInput").ap()
    ec_ext = nc.dram_tensor("ecp", [128, NCH, 4], F32, kind="ExternalInput").ap()
    fc1_ext = nc.dram_tensor("fc1p", [128, NCH, H], F32, kind="ExternalInput").ap()
    fc2_ext = nc.dram_tensor("fc2p", [H, NCH, 128], F32, kind="ExternalInput").ap()
    sel_ext = nc.dram_tensor("selp", [NCH, NCH, 128], BF16, kind="ExternalInput").ap()
    d1_ext = nc.dram_tensor("d1", [128, 3, NCH, 128], BF16, kind="ExternalInput").ap()
    d23_ext = nc.dram_tensor("d23", [128, 7, NCH, 128], BF16, kind="ExternalInput").ap()
    dec_ext = nc.dram_tensor("dec", [128, 4, NCH, 128], BF16, kind="ExternalInput").ap()
    out_ext = nc.dram_tensor("out", [T, C], F32, kind="ExternalOutput").ap()
    if K_DEBUG:
        dbg_xb_ext = nc.dram_tensor("d_xb", [128, NCH, TF], F32,
                                    kind="ExternalOutput").ap()
        dbg_r_ext = nc.dram_tensor("d_r", [128, NCH, TF], F32,
                                   kind="ExternalOutput").ap()
        dbg_c3_ext = nc.dram_tensor("d_c3", [128, NCH, TF], F32,
                                    kind="ExternalOutput").ap()
        dbg_pg_ext = nc.dram_tensor("d_pg", [128, NCH, 8], F32,
                                    kind="ExternalOutput").ap()
        dbg_ga_ext = nc.dram_tensor("d_ga", [128, C], F32,
                                    kind="ExternalOutput").ap()
        dbg_st_ext = nc.dram_tensor("d_st", [128, NT, 2], F32,
                                    kind="ExternalOutput").ap()

    x_src = x_ext.rearrange("(th p) c -> p th c", p=128)
    res_src = res_ext.rearrange("(th p) c -> p th c", p=128)
    out_dst = out_ext.rearrange("(th p) c -> p th c", p=128)

    with tile.TileContext(nc) as tc, ExitStack() as ctx:
        pool = ctx.enter_context(tc.tile_pool(name="main", bufs=1))
        from concourse.tile_rust import add_dep_helper

        # ---- weights ----
        w1sb = pool.tile([128, NCH, 3], F32, tag="w1sb")
        b1sb = pool.tile([128, NCH], F32, tag="b1sb")
        w23sb = pool.tile([128, NCH, 7], F32, tag="w23sb")
        ecsb = pool.tile([128, NCH, 4], F32, tag="ecsb")
        d1sb = pool.tile([128, 3, NCH, 128], BF16, tag="d1sb")
        selsb = pool.tile([NCH, NCH, 128], BF16, tag="selsb")
        fc1sb = pool.tile([128, NCH, H], F32, tag="fc1sb")
        fc2sb = pool.tile([H, NCH, 128], F32, tag="fc2sb")
        # all weight loads are deferred into phase A (scalar HWDGE queue) so
        # they don't starve the x loads at t=0
        d23sb = pool.tile([128, 7, NCH, 128], BF16, tag="d23sb")
        decsb = pool.tile([128, 4, NCH, 128], BF16, tag="decsb")

        def emit_small_weights():
            # tiny loads: complete in ~us, so HWDGE sem-lane collisions with
            # the xbar transposes are harmless
            nc.scalar.dma_start(d1sb[:], d1_ext)
            nc.scalar.dma_start(w1sb[:], w1_ext)
            nc.scalar.dma_start(b1sb[:], b1_ext)
            nc.scalar.dma_start(w23sb[:], w23_ext)
            nc.scalar.dma_start(ecsb[:], ec_ext)
            nc.scalar.dma_start(selsb[:], sel_ext)
            nc.scalar.dma_start(fc1sb[:], fc1_ext)
            nc.scalar.dma_start(fc2sb[:], fc2_ext)

        # ---- buffers ----
        zX = pool.tile([128, NT, C], BF16, tag="zX")
        sums = pool.tile([128, NT], F32, tag="sums")
        sumsq = pool.tile([128, NT], F32, tag="sumsq")
        scr = pool.tile([128, C], BF16, tag="scr")
        scr2 = pool.tile([128, C], BF16, tag="scr2")
        mu = pool.tile([128, NT], F32, tag="mu")
        rstd = pool.tile([128, NT], F32, tag="rstd")
        varv = pool.tile([128, NT], F32, tag="varv")
        epsb = pool.tile([128, 1], F32, tag="epsb")
        nc.vector.memset(epsb[:], EPS)
        # A->B stage: four independent tiles (4 t-tiles each) so the coarse
        # range tracker never aliases adjacent groups' transposes.
        # B->A stage: two independent half tiles (8 t-tiles each).
        stgab = [pool.tile([128, 4 * C], BF16, tag=f"sab{i}", name=f"sab{i}")
                 for i in range(4)]
        stgba = [pool.tile([128, 8 * C], BF16, tag=f"sba{i}", name=f"sba{i}")
                 for i in range(2)]

        def sab(g):  # A->B view for 4-tile group g: [p, th(4), ch, t(128)]
            return stgab[g][:].rearrange("p (th ch t) -> p th ch t",
                                         th=4, ch=NCH)

        def sba(h):  # B->A view of half h: [p, ch, th(8), c(128)]
            return stgba[h][:].rearrange("p (ch th c) -> p ch th c",
                                         ch=NCH, th=8)
        xB = pool.tile([128, NCH, TF], BF16, tag="xB")   # also aliases c3
        nc.vector.memset(xB[:, :, 0:PAD], 0.0)
        nc.vector.memset(xB[:, :, PAD + T:TF], 0.0)
        rall = pool.tile([128, NCH, TF], BF16, tag="rall")
        nc.vector.memset(rall[:, :, 0:PAD], 0.0)
        nc.vector.memset(rall[:, :, PAD + T:TF], 0.0)
        pools = pool.tile([128, NCH, 4], F32, tag="pools")
        gate_bf = pool.tile([128, 128], BF16, tag="gate_bf")
        nc.vector.memset(gate_bf[:], 0.0)
        gTst = pool.tile([128, 128], BF16, tag="gTst")
        gateA = pool.tile([128, C], BF16, tag="gateA")

        def xft(t):  # stats/apply read the cast tile in place
            return zX[:, t, :]

        # ---- x loads: cast f32->bf16 straight into zX, all queued up
        # front on the gpsimd SWDGE queue (no slot reuse -> no coupling)
        for c in range(8):
            nc.gpsimd.dma_start(zX[:, 2 * c:2 * c + 2, :],
                                x_src[:, 2 * c:2 * c + 2, :])

        # ---- phase A: stats -> apply -> xbar; scatters one group behind ----
        NG = NT // 4
        xpose = [None] * NG

        def emit_scatter(g):
            for i in range(4):
                t = 4 * g + i
                dst = xB[:, :, PAD + t * 128:PAD + (t + 1) * 128]
                on_act = (t % 2 == 0) if SCATTER == "split" else \
                    (SCATTER == "act")
                if on_act:
                    cp = nc.scalar.activation(dst, sab(g)[:, i, :, :],
                                              AF.Copy)
                else:
                    cp = nc.vector.tensor_copy(dst, sab(g)[:, i, :, :])
                add_dep_helper(cp.ins, xpose[g].ins, reason="xbar ordering")

        for g in range(NG):
            ts0 = 4 * g
            if g == 1:
                emit_small_weights()
                nc.scalar.dma_start(d23sb[:], d23_ext)
                nc.scalar.dma_start(decsb[:], dec_ext)
            for t in range(ts0, ts0 + 4):
                if t in STATS_ACT:
                    nc.scalar.activation(scr[:], xft(t), AF.Copy,
                                         accum_out=sums[:, t:t + 1])
                    nc.scalar.activation(scr2[:], xft(t), AF.Square,
                                         accum_out=sumsq[:, t:t + 1])
                    nc.vector.tensor_scalar_mul(mu[:, t:t + 1],
                                                sums[:, t:t + 1], 1.0 / C)
                    nc.vector.tensor_tensor(varv[:, t:t + 1], mu[:, t:t + 1],
                                            mu[:, t:t + 1], op=OP.mult)
                    nc.vector.scalar_tensor_tensor(varv[:, t:t + 1],
                                                   sumsq[:, t:t + 1],
                                                   1.0 / C, varv[:, t:t + 1],
                                                   OP.mult, OP.subtract)
                else:
                    bs = pool.tile([128, 2, 6], F32, tag="bstats",
                                   name=f"bs_{t}", bufs=4)
                    nc.vector.bn_stats(bs[:, 0, :], xft(t)[:, 0:512])
                    nc.vector.bn_stats(bs[:, 1, :], xft(t)[:, 512:1024])
                    agg = pool.tile([128, 2], F32, tag="agg",
                                    name=f"agg_{t}", bufs=4)
                    nc.vector.bn_aggr(agg[:], bs[:])
                    nc.vector.tensor_copy(mu[:, t:t + 1], agg[:, 0:1])
                    nc.vector.tensor_copy(varv[:, t:t + 1], agg[:, 1:2])
            gs = slice(ts0, ts0 + 4)
            nc.scalar.activation(varv[:, gs], varv[:, gs], AF.Sqrt,
                                 bias=epsb[:])
            nc.vector.reciprocal(rstd[:, gs], varv[:, gs])
            for t in range(ts0, ts0 + 4):
                nc.vector.tensor_scalar(zX[:, t, :], xft(t),
                                        mu[:, t:t + 1], rstd[:, t:t + 1],
                                        OP.subtract, OP.mult)
            xpose[g] = nc.sync.dma_start(
                out=sab(g),
                in_=zX[:, ts0:ts0 + 4, :].rearrange("p a b -> p (a b)"),
                transpose=True)
            if g > 0:
                emit_scatter(g - 1)
        emit_scatter(NG - 1)

        # residual in (cast f32->bf16, slow path; overlaps the conv phase).
        # reuses zX; must be emitted after the A->B transposes that read it.
        for q in range(8):
            nc.gpsimd.dma_start(zX[:, q * 2:(q + 1) * 2, :],
                                res_src[:, q * 2:(q + 1) * 2, :])

        if K_DEBUG:
            nc.gpsimd.dma_start(dbg_xb_ext[:], xB[:])  # cast bf16->f32
            nc.sync.dma_start(dbg_st_ext[:, :, 0], mu[:])
            nc.sync.dma_start(dbg_st_ext[:, :, 1], rstd[:])

        # ---- phase B: convs, quarter-pipelined ----
        psum = ctx.enter_context(tc.tile_pool(name="ps", bufs=5, space="PSUM"))

        def rsl(ch, a, b):
            return rall[:, ch, a:b]

        def conv1_pe(ch, q):
            lo, hi = W1[q], W1[q + 1]
            n = hi - lo
            ps1 = psum.tile([128, QT], F32, tag="cps", name=f"c1ps_{ch}_{q}")
            for k in range(3):
                off = PAD - 1 + k + lo
                nc.tensor.matmul(ps1[:, 0:n], d1sb[:, k, ch, :],
                                 xB[:, ch, off:off + n],
                                 start=(k == 0), stop=(k == 2))
            nc.scalar.activation(rsl(ch, PAD + lo, PAD + hi), ps1[:, 0:n],
                                 AF.Relu, bias=b1sb[:, ch:ch + 1])

        def conv1_lastcol(ch):
            # final column t=2047 for PE channels (tiny DVE ops)
            sa = rsl(ch, PAD + 2047, PAD + 2048)
            xs = lambda d: xB[:, ch, PAD + 2047 + d:PAD + 2048 + d]
            nc.vector.tensor_scalar(sa, xs(-1), w1sb[:, ch, 0:1], None,
                                    OP.mult)
            nc.vector.scalar_tensor_tensor(sa, xs(0), w1sb[:, ch, 1:2], sa,
                                           OP.mult, OP.add)
            nc.vector.scalar_tensor_tensor(sa, xs(1), w1sb[:, ch, 2:3], sa,
                                           OP.mult, OP.add)
            nc.vector.tensor_scalar(sa, sa, b1sb[:, ch:ch + 1], 0.0,
                                    OP.add, OP.max)

        def conv1_chain(ch, q):
            lo = W1[q]
            hi = 2048 if q == 3 else W1[q + 1]
            a = rsl(ch, PAD + lo, PAD + hi)
            xs = lambda d: xB[:, ch, PAD + lo + d:PAD + hi + d]
            nc.vector.tensor_scalar(a, xs(-1), w1sb[:, ch, 0:1], None,
                                    OP.mult)
            nc.vector.scalar_tensor_tensor(a, xs(0), w1sb[:, ch, 1:2], a,
                                           OP.mult, OP.add)
            nc.vector.scalar_tensor_tensor(a, xs(1), w1sb[:, ch, 2:3], a,
                                           OP.mult, OP.add)
            nc.vector.tensor_scalar(a, a, b1sb[:, ch:ch + 1], 0.0,
                                    OP.add, OP.max)

        def conv23_pe(ch, q):
            lo = W23[q]
            ps2 = psum.tile([128, QT], F32, tag="cps", name=f"c23ps_{ch}_{q}")
            for k in range(7):
                off = PAD - 3 + k + lo
                nc.tensor.matmul(ps2[:], d23sb[:, k, ch, :],
                                 rsl(ch, off, off + QT),
                                 start=(k == 0),
                                 stop=(k == 6 and q not in (0, 3)))
            if q == 0:
                nc.tensor.matmul(ps2[:, 0:1], decsb[:, 0, ch, :],
                                 rsl(ch, PAD, PAD + 1),
                                 start=False, stop=False)
                nc.tensor.matmul(ps2[:, 0:1], decsb[:, 1, ch, :],
                                 rsl(ch, PAD + 1, PAD + 2),
                                 start=False, stop=True)
            elif q == 3:
                nc.tensor.matmul(ps2[:, QT - 1:QT], decsb[:, 2, ch, :],
                                 rsl(ch, PAD + T - 2, PAD + T - 1),
                                 start=False, stop=False)
                nc.tensor.matmul(ps2[:, QT - 1:QT], decsb[:, 3, ch, :],
                                 rsl(ch, PAD + T - 1, PAD + T),
                                 start=False, stop=True)
            nc.scalar.activation(xB[:, ch, PAD + lo:PAD + lo + QT],
                                 ps2[:], AF.Copy,
                                 accum_out=pools[:, ch, q:q + 1])

        def conv23_chain(ch, q):
            lo = W23[q]
            out = xB[:, ch, PAD + lo:PAD + lo + QT]
            rs = lambda d: rsl(ch, PAD + lo - 3 + d, PAD + lo + QT - 3 + d)
            nc.vector.tensor_scalar(out, rs(0), w23sb[:, ch, 0:1], None,
                                    OP.mult)
            if q == 0:
                nc.vector.scalar_tensor_tensor(out[:, 0:1],
                                               rsl(ch, PAD, PAD + 1),
                                               ecsb[:, ch, 0:1], out[:, 0:1],
                                               OP.mult, OP.add)
                nc.vector.scalar_tensor_tensor(out[:, 0:1],
                                               rsl(ch, PAD + 1, PAD + 2),
                                               ecsb[:, ch, 1:2], out[:, 0:1],
                                               OP.mult, OP.add)
            elif q == 3:
                nc.vector.scalar_tensor_tensor(out[:, QT - 1:QT],
                                               rsl(ch, PAD + T - 2,
                                                   PAD + T - 1),
                                               ecsb[:, ch, 2:3],
                                               out[:, QT - 1:QT],
                                               OP.mult, OP.add)
                nc.vector.scalar_tensor_tensor(out[:, QT - 1:QT],
                                               rsl(ch, PAD + T - 1, PAD + T),
                                               ecsb[:, ch, 3:4],
                                               out[:, QT - 1:QT],
                                               OP.mult, OP.add)
            for k in range(1, 7):
                nc.vector.scalar_tensor_tensor(
                    out, rs(k), w23sb[:, ch, k:k + 1], out, OP.mult, OP.add,
                    accum_out=(pools[:, ch, q:q + 1] if k == 6 else None))

        def conv1_any(ch, q):
            if CH_ENG[ch] == 'P':
                conv1_pe(ch, q)
            else:
                conv1_chain(ch, q)

        def conv23_any(ch, q):
            if CH_ENG[ch] == 'V':
                conv23_chain(ch, q)
            else:
                conv23_pe(ch, q)

        # stage-major emission: conv23 trails conv1 by one window
        for ch in range(NCH):
            conv1_any(ch, 0)
        for ch in range(NCH):
            conv1_any(ch, 1)
        for ch in range(NCH):
            conv23_any(ch, 0)
        for ch in range(NCH):
            conv1_any(ch, 2)
        for ch in range(NCH):
            conv23_any(ch, 1)
        for ch in range(NCH):
            conv1_any(ch, 3)
            if CH_ENG[ch] == 'P':
                conv1_lastcol(ch)
        # channel-major for the last stages so each channel's transpose-back
        # starts as soon as that channel is done (overlaps remaining convs)
        for ch in range(NCH):
            conv23_any(ch, 2)
            conv23_any(ch, 3)
            for h in range(2):
                nc.sync.dma_start(out=sba(h)[:, ch, :, :],
                                  in_=xB[:, ch, PAD + 1024 * h:
                                         PAD + 1024 * (h + 1)],
                                  transpose=True)

        if K_DEBUG:
            nc.gpsimd.dma_start(dbg_r_ext[:], rall[:])
            nc.gpsimd.dma_start(dbg_c3_ext[:], xB[:])

        # ---- SE MLP ----
        se_ps = ctx.enter_context(tc.tile_pool(name="seps", bufs=1,
                                               space="PSUM"))
        h_ps = se_ps.tile([H, 4], F32, tag="hps")
        for ch in range(NCH):
            nc.tensor.matmul(h_ps[:], fc1sb[:, ch, :], pools[:, ch, :],
                             start=(ch == 0), stop=(ch == NCH - 1))
        h_half = pool.tile([H, 4], F32, tag="h_half")
        nc.scalar.activation(h_half[:], h_ps[:], AF.Relu)
        h_sb = pool.tile([H, 1], F32, tag="hsb")
        nc.vector.tensor_reduce(h_sb[:], h_half[:], mybir.AxisListType.X,
                                OP.add)
        g_ps = se_ps.tile([128, 1024], F32, tag="gps")
        for ch in range(NCH):
            nc.tensor.matmul(g_ps[:, ch:ch + 1], fc2sb[:, ch, :], h_sb[:],
                             start=True, stop=True)
        nc.scalar.activation(gate_bf[:, 0:NCH], g_ps[:, 0:NCH], AF.Sigmoid)

        # ---- gate -> layout A row, replicated across partitions ----
        nc.sync.dma_start(out=gTst[:], in_=gate_bf[:], transpose=True)
        for ch in range(NCH):
            nc.tensor.matmul(g_ps[:, ch * 128:(ch + 1) * 128],
                             selsb[:, ch, :], gTst[0:NCH, :],
                             start=True, stop=True)
        nc.scalar.activation(gateA[:], g_ps[:], AF.Copy)
        gateA3 = gateA[:].rearrange("p (ch c) -> p ch c", ch=NCH)
        if K_DEBUG:
            nc.sync.dma_start(dbg_pg_ext[:, :, 0:4], pools[:])
            nc.gpsimd.dma_start(dbg_pg_ext[:, :, 4], gate_bf[:, 0:NCH])
            nc.gpsimd.dma_start(dbg_ga_ext[:], gateA[:])

        # ---- output: gate mult + residual add + store, per 2-tile chunk ----
        for q in range(8):
            hf, to = q // 4, (2 * q) % 8
            for t in (2 * q, 2 * q + 1):
                st = sba(hf)[:, :, t % 8, :]
                nc.vector.tensor_tensor(st, st, gateA3, op=OP.mult)
            zt = zX[:, 2 * q:2 * q + 2, :].rearrange(
                "p th (ch c) -> p ch th c", ch=NCH)
            nc.vector.tensor_tensor(zt, zt, sba(hf)[:, :, to:to + 2, :],
                                    op=OP.add)
            nc.gpsimd.dma_start(out_dst[:, 2 * q:2 * q + 2, :],
                                zX[:, 2 * q:2 * q + 2, :])

    nc.compile()
    return nc


def _prep_weights(ln_w, ln_b, w1, w2, w3, fc1, fc2):
    import ml_dtypes
    w1 = w1[:, 0, :].astype(np.float64)   # [C, 3]
    w2 = w2[:, 0, :].astype(np.float64)   # [C, 5]
    w3 = w3[:, 0, :].astype(np.float64)   # [C, 3]
    ln_w = ln_w.astype(np.float64)
    ln_b = ln_b.astype(np.float64)
    w1f = w1 * ln_w[:, None]
    b1 = (ln_b * w1.sum(axis=1))

    def to_plh(a):  # [C, K] -> [128, NCH, K]
        return np.ascontiguousarray(
            a.reshape(NCH, 128, -1).transpose(1, 0, 2)).astype(np.float32)

    w1p = to_plh(w1f)
    b1p = np.ascontiguousarray(b1.reshape(NCH, 128).T).astype(np.float32)
    fc1p = to_plh((fc1.astype(np.float64) / T).T)
    fc2p = np.ascontiguousarray(
        fc2.astype(np.float64).T.reshape(H, NCH, 128)).astype(np.float32)

    w23 = np.stack([np.convolve(w3[c], w2[c]) for c in range(C)])  # [C, 7]
    # edge-fix coefficients (negated: they accumulate into the psum)
    ec = np.stack([-w3[:, 0] * w2[:, 3], -w3[:, 0] * w2[:, 4],
                   -w3[:, 2] * w2[:, 0], -w3[:, 2] * w2[:, 1]], axis=1)  # [C,4]
    w23p = to_plh(w23)
    ecp = to_plh(ec)

    selp = np.zeros((NCH, NCH, 128), np.float32)
    for ch in range(NCH):
        selp[ch, ch, :] = 1.0
    selp = selp.astype(ml_dtypes.bfloat16)

    def diags(wk):  # [C, K] -> [128, K, NCH, 128] bf16 (partition-major)
        K = wk.shape[1]
        d = np.zeros((K, NCH, 128, 128), np.float32)
        for k in range(K):
            for chh in range(NCH):
                np.fill_diagonal(d[k, chh], wk[chh * 128:(chh + 1) * 128, k])
        return np.ascontiguousarray(
            d.transpose(2, 0, 1, 3)).astype(ml_dtypes.bfloat16)

    return {"w1p": w1p, "b1p": b1p, "w23p": w23p, "ecp": ecp,
            "fc1p": fc1p, "fc2p": fc2p, "selp": selp,
            "d1": diags(w1f), "d23": diags(w23), "dec": diags(ec)}


def kernel(x, residual_input, ln_w, ln_b, w1, w2, w3, fc1, fc2):
    x = np.asarray(x, dtype=np.float32)
    residual_input = np.asarray(residual_input, dtype=np.float32)
    wts = _prep_weights(np.asarray(ln_w), np.asarray(ln_b),
                        np.asarray(w1), np.asarray(w2), np.asarray(w3),
                        np.asarray(fc1), np.asarray(fc2))

    if "nc" not in _CACHE:
        _CACHE["nc"] = _build()
    nc = _CACHE["nc"]

    in_maps = []
    for b in range(B):
        m = {"x": np.ascontiguousarray(x[b]),
             "res": np.ascontiguousarray(residual_input[b])}
        m.update(wts)
        in_maps.append(m)
    res = run_bass_kernel_spmd(nc, in_maps, core_ids=list(range(N_CORES)))
    out = np.stack([res.results[i]["out"] for i in range(N_CORES)], axis=0)
    return out.astype(np.float32)



# revision 11
# speedup vs baseline: 1.0667x; 1.0667x over previous
"""Trainium2 Bass kernel for nn_AdapterBlock (LN -> dwconv x3 -> SE -> residual).

Data-parallel over batch: 8 samples -> 8 NeuronCores. v4: weights front-loaded,
k-outer conv23 (weight reuse, 4x fewer LDWEIGHTS), stall-free PE stream,
single-matmul gate broadcast, conv1 spread over PE/DVE/Pool, tail split
DVE/Pool with bf16 HWDGE store (host upcast).

Per core:
  - d1 diag weights load first (conv1 needs them at ~15us), then x (SWDGE
    cast f32->bf16), then the remaining weights
  - LN stats per t-tile split ACT(accum)/DVE(bn_stats); apply on DVE
  - xbar DMA-transpose to layout B per 4-tile group + ACT/DVE re-scatter
  - conv1 windows staggered ([0,511,1023,1535,2047]) so each window only
    needs already-scattered tiles; per-channel engine map CH_ENG:
      P: conv1 diag-matmul on PE; H: conv1 chain on DVE; G: chain on Pool
    conv23 always PE, k-outer over quarter-pairs so each LDWEIGHTS serves
    2 matmuls back-to-back and the PE never waits on a fresh weight stream
  - PE stream order: conv1 q0,q1,q2 -> conv23 q01 (all ch) -> conv1 q3 ->
    conv23 q23 (all ch); keeps PE busy while scatter of group 3 finishes
  - SE pool rides the PSUM evacuation (ACT accum_out)
  - c3 output aliases xB; B->A transpose-back per channel during conv
  - gate broadcast: replicate h over 128 cols, ONE matmul pair with
    fc2 [H, C] bf16 as moving operand -> sigmoid -> gateA (no transpose)
  - tail: out = stg*gateA + residual per chunk, split DVE/Pool; bf16
    HWDGE store on scalar/sync queues, host upcasts to f32
"""

import os
import sys

sys.path.insert(0, "/opt/trn_rl_repo")

from contextlib import ExitStack

import numpy as np

import concourse.bass as bass  # noqa: F401
import concourse.bacc as bacc
import concourse.tile as tile
import concourse.mybir as mybir
from concourse.bass_utils import run_bass_kernel_spmd

B, T, C = 8, 2048, 1024
N_CORES = 8
NT = T // 128          # 16 t-tiles
NCH = C // 128         # 8 channel groups
H = C // 16            # SE hidden = 64
PAD = 4                # zero pad each side of the time axis (>= conv halo 3)
TF = T + 2 * PAD
QT = 512               # conv quarter
EPS = 1e-5

# conv1 windows staggered so window q only reads tiles already scattered
W1 = [0, 511, 1023, 1535, 2047]
W23 = [0, 512, 1024, 1536, 2048]

F32 = mybir.dt.float32
BF16 = mybir.dt.bfloat16
AF = mybir.ActivationFunctionType
OP = mybir.AluOpType

# --- tunables ------------------------------------------------------------
# engine per channel-group for conv1: 'P' = TensorE diag-matmul,
# 'H' = DVE chain. conv23 is always PE. P channels must form a prefix.
CH_ENG = os.environ.get("K_CH_ENG", "PPPHHHHH")
SCATTER = os.environ.get("K_SCATTER", "split")  # split | dve | act
STATS_ACT = set(int(x) for x in
                os.environ.get("K_STATS_ACT", "0,2,4,6").split(",")
                if x != "")
# tail chunk engines: 'V' = DVE, 'G' = Pool (gpsimd TensorTensor)
TAIL_ENG = os.environ.get("K_TAIL_ENG", "VVVVVVVV")
N_P = len(CH_ENG) - len(CH_ENG.lstrip('P'))  # leading P channels

_CACHE = {}


def _build():
    nc = bacc.Bacc("TRN2", target_bir_lowering=False, debug=False,
                   num_devices=N_CORES)

    x_ext = nc.dram_tensor("x", [T, C], F32, kind="ExternalInput").ap()
    res_ext = nc.dram_tensor("res", [T, C], F32, kind="ExternalInput").ap()
    w1_ext = nc.dram_tensor("w1p", [128, NCH, 3], F32, kind="ExternalInput").ap()
    b1_ext = nc.dram_tensor("b1p", [128, NCH], F32, kind="ExternalInput").ap()
    w23_ext = nc.dram_tensor("w23p", [128, NCH, 7], F32, kind="ExternalInput").ap()
    ec_ext = nc.dram_tensor("ecp", [128, NCH, 4], F32, kind="ExternalInput").ap()
    fc1_ext = nc.dram_tensor("fc1p", [128, NCH, H], F32, kind="ExternalInput").ap()
    fc2_ext = nc.dram_tensor("fc2p", [H, NCH * 128], BF16, kind="ExternalInput").ap()
    d1_ext = nc.dram_tensor("d1", [128, 3, NCH, 128], BF16, kind="ExternalInput").ap()
    d23_ext = nc.dram_tensor("d23", [128, 7, NCH, 128], BF16, kind="ExternalInput").ap()
    out_ext = nc.dram_tensor("out", [T, C], BF16, kind="ExternalOutput").ap()

    x_src = x_ext.rearrange("(th p) c -> p th c", p=128)
    res_src = res_ext.rearrange("(th p) c -> p th c", p=128)
    out_dst = out_ext.rearrange("(th p) c -> p th c", p=128)

    with tile.TileContext(nc) as tc, ExitStack() as ctx:
        pool = ctx.enter_context(tc.tile_pool(name="main", bufs=1))
        from concourse.tile_rust import add_dep_helper

        # ---- weights ----
        w1sb = pool.tile([128, NCH, 3], F32, tag="w1sb")
        b1sb = pool.tile([128, NCH], F32, tag="b1sb")
        w23sb = pool.tile([128, NCH, 7], F32, tag="w23sb")
        ecsb = pool.tile([128, NCH, 4], F32, tag="ecsb")
        d1sb = pool.tile([128, 3, NCH, 128], BF16, tag="d1sb")
        fc1sb = pool.tile([128, NCH, H], F32, tag="fc1sb")
        fc2sb = pool.tile([H, NCH * 128], BF16, tag="fc2sb")
        d23sb = pool.tile([128, 7, NCH, 128], BF16, tag="d23sb")

        # conv1 diag weights first: conv1 on PE is the first consumer (~15us);
        # only the P channels need the diag form
        if N_P:
            nc.scalar.dma_start(d1sb[:, :, 0:N_P, :], d1_ext[:, :, 0:N_P, :])

        # ---- buffers ----
        zX = pool.tile([128, NT, C], BF16, tag="zX")
        sums = pool.tile([128, NT], F32, tag="sums")
        sumsq = pool.tile([128, NT], F32, tag="sumsq")
        scr = pool.tile([128, C], BF16, tag="scr")
        scr2 = pool.tile([128, C], BF16, tag="scr2")
        mu = pool.tile([128, NT], F32, tag="mu")
        rstd = pool.tile([128, NT], F32, tag="rstd")
        varv = pool.tile([128, NT], F32, tag="varv")
        epsb = pool.tile([128, 1], F32, tag="epsb")
        nc.vector.memset(epsb[:], EPS)
        # A->B stage: four independent tiles (4 t-tiles each) so the coarse
        # range tracker never aliases adjacent groups' transposes.
        # B->A stage: two independent half tiles (8 t-tiles each).
        stgab = [pool.tile([128, 4 * C], BF16, tag=f"sab{i}", name=f"sab{i}")
                 for i in range(4)]
        stgba = [pool.tile([128, 8 * C], BF16, tag=f"sba{i}", name=f"sba{i}")
                 for i in range(2)]

        def sab(g):  # A->B view for 4-tile group g: [p, th(4), ch, t(128)]
            return stgab[g][:].rearrange("p (th ch t) -> p th ch t",
                                         th=4, ch=NCH)

        def sba(h):  # B->A view of half h: [p, ch, th(8), c(128)]
            return stgba[h][:].rearrange("p (ch th c) -> p ch th c",
                                         ch=NCH, th=8)
        xB = pool.tile([128, NCH, TF], BF16, tag="xB")   # also aliases c3
        nc.vector.memset(xB[:, :, 0:PAD], 0.0)
        nc.vector.memset(xB[:, :, PAD + T:TF], 0.0)
        rall = pool.tile([128, NCH, TF], BF16, tag="rall")
        nc.vector.memset(rall[:, :, 0:PAD], 0.0)
        nc.vector.memset(rall[:, :, PAD + T:TF], 0.0)
        pools = pool.tile([128, NCH, 4], F32, tag="pools")
        gateA = pool.tile([128, C], BF16, tag="gateA")
        h_rep = pool.tile([H, 128], BF16, tag="h_rep")
        ones_h = pool.tile([H, 128], BF16, tag="ones_h")
        nc.vector.memset(ones_h[:], 1.0)

        def xft(t):  # stats/apply read the cast tile in place
            return zX[:, t, :]

        # ---- x loads: cast f32->bf16 straight into zX, all queued up
        # front on the gpsimd SWDGE queue (no slot reuse -> no coupling)
        for c in range(8):
            nc.gpsimd.dma_start(zX[:, 2 * c:2 * c + 2, :],
                                x_src[:, 2 * c:2 * c + 2, :])

        # remaining weights (scalar HWDGE queue, parallel to the x stream);
        # dec/d23 are needed by conv23 which starts ~30us in
        nc.scalar.dma_start(w1sb[:], w1_ext)
        nc.scalar.dma_start(b1sb[:], b1_ext)
        nc.scalar.dma_start(w23sb[:], w23_ext)
        nc.scalar.dma_start(ecsb[:], ec_ext)
        nc.scalar.dma_start(fc1sb[:], fc1_ext)
        nc.scalar.dma_start(fc2sb[:], fc2_ext)
        nc.scalar.dma_start(d23sb[:], d23_ext)

        # ---- phase A: stats -> apply -> xbar; scatters one group behind ----
        NG = NT // 4
        xpose = [None] * NG

        def emit_scatter(g):
            for i in range(4):
                t = 4 * g + i
                dst = xB[:, :, PAD + t * 128:PAD + (t + 1) * 128]
                on_act = (t % 2 == 0) if SCATTER == "split" else \
                    (SCATTER == "act")
                if on_act:
                    cp = nc.scalar.activation(dst, sab(g)[:, i, :, :],
                                              AF.Copy)
                else:
                    cp = nc.vector.tensor_copy(dst, sab(g)[:, i, :, :])
                add_dep_helper(cp.ins, xpose[g].ins, reason="xbar ordering")

        for g in range(NG):
            ts0 = 4 * g
            for t in range(ts0, ts0 + 4):
                if t in STATS_ACT:
                    nc.scalar.activation(scr[:], xft(t), AF.Copy,
                                         accum_out=sums[:, t:t + 1])
                    nc.scalar.activation(scr2[:], xft(t), AF.Square,
                                         accum_out=sumsq[:, t:t + 1])
                    nc.vector.tensor_scalar_mul(mu[:, t:t + 1],
                                                sums[:, t:t + 1], 1.0 / C)
                    nc.vector.tensor_tensor(varv[:, t:t + 1], mu[:, t:t + 1],
                                            mu[:, t:t + 1], op=OP.mult)
                    nc.vector.scalar_tensor_tensor(varv[:, t:t + 1],
                                                   sumsq[:, t:t + 1],
                                                   1.0 / C, varv[:, t:t + 1],
                                                   OP.mult, OP.subtract)
                else:
                    bs = pool.tile([128, 2, 6], F32, tag="bstats",
                                   name=f"bs_{t}", bufs=4)
                    nc.vector.bn_stats(bs[:, 0, :], xft(t)[:, 0:512])
                    nc.vector.bn_stats(bs[:, 1, :], xft(t)[:, 512:1024])
                    agg = pool.tile([128, 2], F32, tag="agg",
                                    name=f"agg_{t}", bufs=4)
                    nc.vector.bn_aggr(agg[:], bs[:])
                    nc.vector.tensor_copy(mu[:, t:t + 1], agg[:, 0:1])
                    nc.vector.tensor_copy(varv[:, t:t + 1], agg[:, 1:2])
            gs = slice(ts0, ts0 + 4)
            nc.scalar.activation(varv[:, gs], varv[:, gs], AF.Sqrt,
                                 bias=epsb[:])
            nc.vector.reciprocal(rstd[:, gs], varv[:, gs])
            for t in range(ts0, ts0 + 4):
                nc.vector.tensor_scalar(zX[:, t, :], xft(t),
                                        mu[:, t:t + 1], rstd[:, t:t + 1],
                                        OP.subtract, OP.mult)
            xpose[g] = nc.sync.dma_start(
                out=sab(g),
                in_=zX[:, ts0:ts0 + 4, :].rearrange("p a b -> p (a b)"),
                transpose=True)
            if g > 0:
                emit_scatter(g - 1)
        emit_scatter(NG - 1)

        # residual in (cast f32->bf16, slow path; overlaps the conv phase).
        # reuses zX; must be emitted after the A->B transposes that read it.
        for q in range(8):
            nc.gpsimd.dma_start(zX[:, q * 2:(q + 1) * 2, :],
                                res_src[:, q * 2:(q + 1) * 2, :])

        # ---- phase B: convs ----
        psum = ctx.enter_context(tc.tile_pool(name="ps", bufs=5, space="PSUM"))

        def rsl(ch, a, b):
            return rall[:, ch, a:b]

        def conv1_pe(ch, q):
            lo, hi = W1[q], W1[q + 1]
            n = hi - lo
            ps1 = psum.tile([128, QT], F32, tag="cps", name=f"c1ps_{ch}_{q}")
            for k in range(3):
                off = PAD - 1 + k + lo
                nc.tensor.matmul(ps1[:, 0:n], d1sb[:, k, ch, :],
                                 xB[:, ch, off:off + n],
                                 start=(k == 0), stop=(k == 2))
            nc.scalar.activation(rsl(ch, PAD + lo, PAD + hi), ps1[:, 0:n],
                                 AF.Relu, bias=b1sb[:, ch:ch + 1])

        def conv1_lastcol(ch):
            # final column t=2047 for PE channels (tiny DVE ops)
            sa = rsl(ch, PAD + 2047, PAD + 2048)
            xs = lambda d: xB[:, ch, PAD + 2047 + d:PAD + 2048 + d]
            nc.vector.tensor_scalar(sa, xs(-1), w1sb[:, ch, 0:1], None,
                                    OP.mult)
            nc.vector.scalar_tensor_tensor(sa, xs(0), w1sb[:, ch, 1:2], sa,
                                           OP.mult, OP.add)
            nc.vector.scalar_tensor_tensor(sa, xs(1), w1sb[:, ch, 2:3], sa,
                                           OP.mult, OP.add)
            nc.vector.tensor_scalar(sa, sa, b1sb[:, ch:ch + 1], 0.0,
                                    OP.add, OP.max)

        def conv1_chain(eng, ch, q):
            lo = W1[q]
            hi = 2048 if q == 3 else W1[q + 1]
            a = rsl(ch, PAD + lo, PAD + hi)
            xs = lambda d: xB[:, ch, PAD + lo + d:PAD + hi + d]
            eng.tensor_scalar(a, xs(-1), w1sb[:, ch, 0:1], None,
                              OP.mult)
            eng.scalar_tensor_tensor(a, xs(0), w1sb[:, ch, 1:2], a,
                                     OP.mult, OP.add)
            eng.scalar_tensor_tensor(a, xs(1), w1sb[:, ch, 2:3], a,
                                     OP.mult, OP.add)
            eng.tensor_scalar(a, a, b1sb[:, ch:ch + 1], 0.0,
                              OP.add, OP.max)

        def conv23_kouter(ch, qs):
            # k-outer over a quarter pair: each LDWEIGHTS serves len(qs)
            # back-to-back matmuls
            pss = {}
            for q in qs:
                pss[q] = psum.tile([128, QT], F32, tag="cps",
                                   name=f"c23ps_{ch}_{q}")
            for k in range(7):
                for q in qs:
                    lo = W23[q]
                    off = PAD - 3 + k + lo
                    nc.tensor.matmul(pss[q][:], d23sb[:, k, ch, :],
                                     rsl(ch, off, off + QT),
                                     start=(k == 0), stop=(k == 6))
            # edge-fix: the padded 7-tap composite differs from the true
            # conv3(conv2(.)) at the two outermost columns; patch the psum
            # with tiny DVE MACs before evacuation
            if 0 in qs:
                e = pss[0][:, 0:1]
                nc.vector.scalar_tensor_tensor(e, rsl(ch, PAD, PAD + 1),
                                               ecsb[:, ch, 0:1], e,
                                               OP.mult, OP.add)
                nc.vector.scalar_tensor_tensor(e, rsl(ch, PAD + 1, PAD + 2),
                                               ecsb[:, ch, 1:2], e,
                                               OP.mult, OP.add)
            if 3 in qs:
                e = pss[3][:, QT - 1:QT]
                nc.vector.scalar_tensor_tensor(e, rsl(ch, PAD + T - 2,
                                                      PAD + T - 1),
                                               ecsb[:, ch, 2:3], e,
                                               OP.mult, OP.add)
                nc.vector.scalar_tensor_tensor(e, rsl(ch, PAD + T - 1,
                                                      PAD + T),
                                               ecsb[:, ch, 3:4], e,
                                               OP.mult, OP.add)
            for q in qs:
                lo = W23[q]
                nc.scalar.activation(xB[:, ch, PAD + lo:PAD + lo + QT],
                                     pss[q][:], AF.Copy,
                                     accum_out=pools[:, ch, q:q + 1])

        def conv1_any(ch, q):
            if CH_ENG[ch] == 'P':
                conv1_pe(ch, q)
            else:
                conv1_chain(nc.vector, ch, q)

        # PE stream: conv1 q0..q2, conv23 q01 (fills the wait for scatter
        # g3), conv1 q3, conv23 q23 (with B->A transpose-back per channel)
        for ch in range(NCH):
            conv1_any(ch, 0)
        for ch in range(NCH):
            conv1_any(ch, 1)
        for ch in range(NCH):
            conv1_any(ch, 2)
        for ch in range(NCH):
            conv23_kouter(ch, (0, 1))
        for ch in range(NCH):
            conv1_any(ch, 3)
            if CH_ENG[ch] == 'P':
                conv1_lastcol(ch)
        # channel-major for the last stages so each channel's transpose-back
        # starts as soon as that channel is done (overlaps remaining convs)
        for ch in range(NCH):
            conv23_kouter(ch, (2, 3))
            for h in range(2):
                nc.sync.dma_start(out=sba(h)[:, ch, :, :],
                                  in_=xB[:, ch, PAD + 1024 * h:
                                         PAD + 1024 * (h + 1)],
                                  transpose=True)

        # ---- SE MLP ----
        se_ps = ctx.enter_context(tc.tile_pool(name="seps", bufs=1,
                                               space="PSUM"))
        h_ps = se_ps.tile([H, 4], F32, tag="hps")
        for ch in range(NCH):
            nc.tensor.matmul(h_ps[:], fc1sb[:, ch, :], pools[:, ch, :],
                             start=(ch == 0), stop=(ch == NCH - 1))
        h_half = pool.tile([H, 4], F32, tag="h_half")
        nc.scalar.activation(h_half[:], h_ps[:], AF.Relu)
        h_sb = pool.tile([H, 1], F32, tag="hsb")
        nc.vector.tensor_reduce(h_sb[:], h_half[:], mybir.AxisListType.X,
                                OP.add)
        # replicate h across 128 cols (per-partition scalar mult of a ones
        # tile), then one matmul pair computes sigmoid(fc2^T h) broadcast
        # over all partitions
        nc.vector.tensor_scalar_mul(h_rep[:], ones_h[:], h_sb[:])
        g_ps = se_ps.tile([128, 1024], F32, tag="gps")
        nc.tensor.matmul(g_ps[:, 0:512], h_rep[:], fc2sb[:, 0:512],
                         start=True, stop=True)
        nc.tensor.matmul(g_ps[:, 512:1024], h_rep[:], fc2sb[:, 512:1024],
                         start=True, stop=True)
        nc.scalar.activation(gateA[:], g_ps[:], AF.Sigmoid)
        gateA3 = gateA[:].rearrange("p (ch c) -> p ch c", ch=NCH)

        # ---- output: gate mult + residual add + store, per 2-tile chunk;
        # work split DVE/Pool, bf16 store on scalar/sync HWDGE queues ----
        for q in range(8):
            hf, to = q // 4, (2 * q) % 8
            eng = nc.gpsimd if TAIL_ENG[q] == 'G' else nc.vector
            for t in (2 * q, 2 * q + 1):
                st = sba(hf)[:, :, t % 8, :]
                eng.tensor_tensor(st, st, gateA3, op=OP.mult)
            zt = zX[:, 2 * q:2 * q + 2, :].rearrange(
                "p th (ch c) -> p ch th c", ch=NCH)
            eng.tensor_tensor(zt, zt, sba(hf)[:, :, to:to + 2, :],
                              op=OP.add)
            dq = nc.scalar if q % 2 == 0 else nc.sync
            dq.dma_start(out_dst[:, 2 * q:2 * q + 2, :],
                         zX[:, 2 * q:2 * q + 2, :])

    nc.compile()
    return nc


def _prep_weights(ln_w, ln_b, w1, w2, w3, fc1, fc2):
    import ml_dtypes
    w1 = w1[:, 0, :].astype(np.float64)   # [C, 3]
    w2 = w2[:, 0, :].astype(np.float64)   # [C, 5]
    w3 = w3[:, 0, :].astype(np.float64)   # [C, 3]
    ln_w = ln_w.astype(np.float64)
    ln_b = ln_b.astype(np.float64)
    w1f = w1 * ln_w[:, None]
    b1 = (ln_b * w1.sum(axis=1))

    def to_plh(a):  # [C, K] -> [128, NCH, K]
        return np.ascontiguousarray(
            a.reshape(NCH, 128, -1).transpose(1, 0, 2)).astype(np.float32)

    w1p = to_plh(w1f)
    b1p = np.ascontiguousarray(b1.reshape(NCH, 128).T).astype(np.float32)
    fc1p = to_plh((fc1.astype(np.float64) / T).T)
    fc2p = np.ascontiguousarray(
        fc2.astype(np.float64).T.reshape(H, NCH * 128)).astype(
            ml_dtypes.bfloat16)

    w23 = np.stack([np.convolve(w3[c], w2[c]) for c in range(C)])  # [C, 7]
    # edge-fix coefficients (negated: they accumulate into the psum)
    ec = np.stack([-w3[:, 0] * w2[:, 3], -w3[:, 0] * w2[:, 4],
                   -w3[:, 2] * w2[:, 0], -w3[:, 2] * w2[:, 1]], axis=1)  # [C,4]
    w23p = to_plh(w23)
    ecp = to_plh(ec)

    def diags(wk):  # [C, K] -> [128, K, NCH, 128] bf16 (partition-major)
        K = wk.shape[1]
        d = np.zeros((K, NCH, 128, 128), np.float32)
        for k in range(K):
            for chh in range(NCH):
                np.fill_diagonal(d[k, chh], wk[chh * 128:(chh + 1) * 128, k])
        return np.ascontiguousarray(
            d.transpose(2, 0, 1, 3)).astype(ml_dtypes.bfloat16)

    return {"w1p": w1p, "b1p": b1p, "w23p": w23p, "ecp": ecp,
            "fc1p": fc1p, "fc2p": fc2p,
            "d1": diags(w1f), "d23": diags(w23)}


def kernel(x, residual_input, ln_w, ln_b, w1, w2, w3, fc1, fc2):
    x = np.asarray(x, dtype=np.float32)
    residual_input = np.asarray(residual_input, dtype=np.float32)
    wts = _prep_weights(np.asarray(ln_w), np.asarray(ln_b),
                        np.asarray(w1), np.asarray(w2), np.asarray(w3),
                        np.asarray(fc1), np.asarray(fc2))

    if "nc" not in _CACHE:
        _CACHE["nc"] = _build()
    nc = _CACHE["nc"]

    in_maps = []
    for b in range(B):
        m = {"x": np.ascontiguousarray(x[b]),
             "res": np.ascontiguousarray(residual_input[b])}
        m.update(wts)
        in_maps.append(m)
    res = run_bass_kernel_spmd(nc, in_maps, core_ids=list(range(N_CORES)))
    out = np.stack([res.results[i]["out"] for i in range(N_CORES)], axis=0)
    return out.astype(np.float32)


# revision 23
# speedup vs baseline: 1.0771x; 1.0097x over previous
"""Trainium2 Bass kernel for nn_AdapterBlock (LN -> dwconv x3 -> SE -> residual).

Data-parallel over batch: 8 samples -> 8 NeuronCores. v4: weights front-loaded,
k-outer conv23 (weight reuse, 4x fewer LDWEIGHTS), stall-free PE stream,
single-matmul gate broadcast, conv1 spread over PE/DVE/Pool, tail split
DVE/Pool with bf16 HWDGE store (host upcast).

Per core:
  - d1 diag weights load first (conv1 needs them at ~15us), then x (SWDGE
    cast f32->bf16), then the remaining weights
  - LN stats per t-tile split ACT(accum)/DVE(bn_stats); apply on DVE
  - xbar DMA-transpose to layout B per 4-tile group + ACT/DVE re-scatter
  - conv1 windows staggered ([0,511,1023,1535,2047]) so each window only
    needs already-scattered tiles; per-channel engine map CH_ENG:
      P: conv1 diag-matmul on PE; H: conv1 chain on DVE; G: chain on Pool
    conv23 always PE, k-outer over quarter-pairs so each LDWEIGHTS serves
    2 matmuls back-to-back and the PE never waits on a fresh weight stream
  - PE stream order: conv1 q0,q1,q2 -> conv23 q01 (all ch) -> conv1 q3 ->
    conv23 q23 (all ch); keeps PE busy while scatter of group 3 finishes
  - SE pool rides the PSUM evacuation (ACT accum_out)
  - c3 output aliases xB; B->A transpose-back per channel during conv
  - gate broadcast: replicate h over 128 cols, ONE matmul pair with
    fc2 [H, C] bf16 as moving operand -> sigmoid -> gateA (no transpose)
  - tail: out = stg*gateA + residual per chunk, split DVE/Pool; bf16
    HWDGE store on scalar/sync queues, host upcasts to f32
"""

import os
import sys

sys.path.insert(0, "/opt/trn_rl_repo")

from contextlib import ExitStack

import numpy as np

import concourse.bass as bass  # noqa: F401
import concourse.bacc as bacc
import concourse.tile as tile
import concourse.mybir as mybir
from concourse.bass_utils import run_bass_kernel_spmd

B, T, C = 8, 2048, 1024
N_CORES = 8
NT = T // 128          # 16 t-tiles
NCH = C // 128         # 8 channel groups
H = C // 16            # SE hidden = 64
PAD = 4                # zero pad each side of the time axis (>= conv halo 3)
TF = T + 2 * PAD
QT = 512               # conv quarter
EPS = 1e-5

# conv1 windows staggered so window q only reads tiles already scattered
W1 = [0, 511, 1023, 1535, 2047]
W23 = [0, 512, 1024, 1536, 2048]

F32 = mybir.dt.float32
BF16 = mybir.dt.bfloat16
AF = mybir.ActivationFunctionType
OP = mybir.AluOpType

# --- tunables ------------------------------------------------------------
# engine per channel-group for conv1: 'P' = TensorE diag-matmul,
# 'H' = DVE chain. conv23 is always PE. P channels must form a prefix.
CH_ENG = os.environ.get("K_CH_ENG", "PPPHHHHH")
SCATTER = os.environ.get("K_SCATTER", "split")  # split | dve | act
STATS_ACT = set(int(x) for x in
                os.environ.get("K_STATS_ACT", "0,2,4,6").split(",")
                if x != "")
# tail chunk engines: 'V' = DVE, 'G' = Pool (gpsimd TensorTensor)
TAIL_ENG = os.environ.get("K_TAIL_ENG", "VVVVVVVV")
N_P = len(CH_ENG) - len(CH_ENG.lstrip('P'))  # leading P channels

_CACHE = {}


def _build():
    nc = bacc.Bacc("TRN2", target_bir_lowering=False, debug=False,
                   num_devices=N_CORES)

    x_ext = nc.dram_tensor("x", [T, C], F32, kind="ExternalInput").ap()
    res_ext = nc.dram_tensor("res", [T, C], F32, kind="ExternalInput").ap()
    w1_ext = nc.dram_tensor("w1p", [128, NCH, 3], F32, kind="ExternalInput").ap()
    b1_ext = nc.dram_tensor("b1p", [128, NCH], F32, kind="ExternalInput").ap()
    w23_ext = nc.dram_tensor("w23p", [128, NCH, 7], F32, kind="ExternalInput").ap()
    ec_ext = nc.dram_tensor("ecp", [128, NCH, 4], F32, kind="ExternalInput").ap()
    fc1_ext = nc.dram_tensor("fc1p", [128, NCH, H], F32, kind="ExternalInput").ap()
    fc2_ext = nc.dram_tensor("fc2p", [H, NCH * 128], BF16, kind="ExternalInput").ap()
    d1_ext = nc.dram_tensor("d1", [128, 3, NCH, 128], BF16, kind="ExternalInput").ap()
    d23_ext = nc.dram_tensor("d23", [128, 7, NCH, 128], BF16, kind="ExternalInput").ap()
    out_ext = nc.dram_tensor("out", [T, C], BF16, kind="ExternalOutput").ap()

    x_src = x_ext.rearrange("(th p) c -> p th c", p=128)
    res_src = res_ext.rearrange("(th p) c -> p th c", p=128)
    out_dst = out_ext.rearrange("(th p) c -> p th c", p=128)

    with tile.TileContext(nc) as tc, ExitStack() as ctx:
        pool = ctx.enter_context(tc.tile_pool(name="main", bufs=1))
        from concourse.tile_rust import add_dep_helper

        # ---- weights ----
        w1sb = pool.tile([128, NCH, 3], F32, tag="w1sb")
        b1sb = pool.tile([128, NCH], F32, tag="b1sb")
        w23sb = pool.tile([128, NCH, 7], F32, tag="w23sb")
        ecsb = pool.tile([128, NCH, 4], F32, tag="ecsb")
        d1sb = pool.tile([128, 3, max(N_P, 1), 128], BF16, tag="d1sb")
        fc1sb = pool.tile([128, NCH, H], F32, tag="fc1sb")
        fc2sb = pool.tile([H, NCH * 128], BF16, tag="fc2sb")
        d23sb = pool.tile([128, 7, NCH, 128], BF16, tag="d23sb")

        # conv1 diag weights first: conv1 on PE is the first consumer (~15us);
        # only the P channels need the diag form
        if N_P:
            nc.scalar.dma_start(d1sb[:, :, 0:N_P, :],
                                d1_ext[:, :, 0:N_P, :])

        # ---- buffers ----
        zX = pool.tile([128, NT, C], BF16, tag="zX")
        sums = pool.tile([128, NT], F32, tag="sums")
        sumsq = pool.tile([128, NT], F32, tag="sumsq")
        scr = pool.tile([128, C], BF16, tag="scr")
        mu = pool.tile([128, NT], F32, tag="mu")
        rstd = pool.tile([128, NT], F32, tag="rstd")
        varv = pool.tile([128, NT], F32, tag="varv")
        epsb = pool.tile([128, 1], F32, tag="epsb")
        nc.vector.memset(epsb[:], EPS)
        # A->B stage: ring of 2 buffers (4 t-tiles each); scatter trails one
        # group behind the transpose so depth 2 suffices.
        # B->A stage: two independent half tiles (8 t-tiles each).
        stgab = [pool.tile([128, 4 * C], BF16, tag="sab", name=f"sab{i}",
                           bufs=2)
                 for i in range(4)]
        stgba = [pool.tile([128, 8 * C], BF16, tag=f"sba{i}", name=f"sba{i}")
                 for i in range(2)]
        # conv23 output, contiguous (ch, t) per half so the B->A transpose
        # reads one flat 2D block
        c3h = [pool.tile([128, NCH * 1024], BF16, tag=f"c3h{i}",
                         name=f"c3h{i}")
               for i in range(2)]

        def sab(g):  # A->B view for 4-tile group g: [p, th(4), ch, t(128)]
            return stgab[g][:].rearrange("p (th ch t) -> p th ch t",
                                         th=4, ch=NCH)

        def sba(h):  # B->A view of half h: [p, ch, th(8), c(128)]
            return stgba[h][:].rearrange("p (ch th c) -> p ch th c",
                                         ch=NCH, th=8)
        xB = pool.tile([128, NCH, TF], BF16, tag="xB")
        nc.vector.memset(xB[:, :, 0:PAD], 0.0)
        nc.vector.memset(xB[:, :, PAD + T:TF], 0.0)
        rall = pool.tile([128, NCH, TF], BF16, tag="rall")
        nc.vector.memset(rall[:, :, 0:PAD], 0.0)
        nc.vector.memset(rall[:, :, PAD + T:TF], 0.0)
        pools = pool.tile([128, NCH, 4], F32, tag="pools")
        gateA = pool.tile([128, C], BF16, tag="gateA")
        h_rep = pool.tile([H, 128], BF16, tag="h_rep")
        ones_h = pool.tile([H, 128], BF16, tag="ones_h")
        nc.vector.memset(ones_h[:], 1.0)

        def xft(t):  # stats/apply read the cast tile in place
            return zX[:, t, :]

        # ---- x loads: cast f32->bf16 straight into zX on the gpsimd SWDGE
        # queue (the only engine that can cast); per-tile granularity so the
        # first stats fire as early as possible
        for t in range(NT):
            nc.gpsimd.dma_start(zX[:, t:t + 1, :], x_src[:, t:t + 1, :])

        # remaining weights (scalar HWDGE queue, parallel to the x stream);
        # dec/d23 are needed by conv23 which starts ~30us in
        nc.scalar.dma_start(w1sb[:], w1_ext)
        nc.scalar.dma_start(b1sb[:], b1_ext)
        nc.scalar.dma_start(w23sb[:], w23_ext)
        nc.scalar.dma_start(ecsb[:], ec_ext)
        nc.scalar.dma_start(fc1sb[:], fc1_ext)
        nc.scalar.dma_start(fc2sb[:], fc2_ext)
        nc.scalar.dma_start(d23sb[:], d23_ext)

        # ---- phase A: stats -> apply -> xbar; scatters one group behind ----
        NG = NT // 4
        xpose = [None] * NG

        def emit_scatter(g):
            # per-(group, channel) copies: the dst range of each copy is
            # contained inside one channel's row of xB, so a conv reading
            # channel ch never picks up false deps on other channels
            for ch in range(NCH):
                dst = xB[:, ch, PAD + g * 512:PAD + (g + 1) * 512].rearrange(
                    "p (th t) -> p th t", th=4)
                src = sab(g)[:, :, ch, :]
                on_act = (ch % 2 == 0) if SCATTER == "split" else \
                    (SCATTER == "act")
                if on_act:
                    cp = nc.scalar.activation(dst, src, AF.Copy)
                else:
                    cp = nc.vector.tensor_copy(dst, src)
                add_dep_helper(cp.ins, xpose[g].ins, reason="xbar ordering")

        for g in range(NG):
            ts0 = 4 * g
            for t in range(ts0, ts0 + 4):
                if t in STATS_ACT:
                    nc.scalar.activation(scr[:], xft(t), AF.Copy,
                                         accum_out=sums[:, t:t + 1])
                    nc.scalar.activation(scr[:], xft(t), AF.Square,
                                         accum_out=sumsq[:, t:t + 1])
                    nc.vector.tensor_scalar_mul(mu[:, t:t + 1],
                                                sums[:, t:t + 1], 1.0 / C)
                    nc.vector.tensor_tensor(varv[:, t:t + 1], mu[:, t:t + 1],
                                            mu[:, t:t + 1], op=OP.mult)
                    nc.vector.scalar_tensor_tensor(varv[:, t:t + 1],
                                                   sumsq[:, t:t + 1],
                                                   1.0 / C, varv[:, t:t + 1],
                                                   OP.mult, OP.subtract)
                else:
                    bs = pool.tile([128, 2, 6], F32, tag="bstats",
                                   name=f"bs_{t}", bufs=4)
                    nc.vector.bn_stats(bs[:, 0, :], xft(t)[:, 0:512])
                    nc.vector.bn_stats(bs[:, 1, :], xft(t)[:, 512:1024])
                    agg = pool.tile([128, 2], F32, tag="agg",
                                    name=f"agg_{t}", bufs=4)
                    nc.vector.bn_aggr(agg[:], bs[:])
                    nc.vector.tensor_copy(mu[:, t:t + 1], agg[:, 0:1])
                    nc.vector.tensor_copy(varv[:, t:t + 1], agg[:, 1:2])
            gs = slice(ts0, ts0 + 4)
            nc.scalar.activation(varv[:, gs], varv[:, gs], AF.Sqrt,
                                 bias=epsb[:])
            nc.vector.reciprocal(rstd[:, gs], varv[:, gs])
            for t in range(ts0, ts0 + 4):
                nc.vector.tensor_scalar(zX[:, t, :], xft(t),
                                        mu[:, t:t + 1], rstd[:, t:t + 1],
                                        OP.subtract, OP.mult)
            xpose[g] = nc.sync.dma_start(
                out=sab(g),
                in_=zX[:, ts0:ts0 + 4, :].rearrange("p a b -> p (a b)"),
                transpose=True)
            if g > 0:
                emit_scatter(g - 1)
        emit_scatter(NG - 1)

        # residual in (cast f32->bf16, slow path; overlaps the conv phase).
        # reuses zX; must be emitted after the A->B transposes that read it.
        for q in range(8):
            nc.gpsimd.dma_start(zX[:, q * 2:(q + 1) * 2, :],
                                res_src[:, q * 2:(q + 1) * 2, :])

        # ---- phase B: convs ----
        psum = ctx.enter_context(tc.tile_pool(name="ps", bufs=5, space="PSUM"))

        def rsl(ch, a, b):
            return rall[:, ch, a:b]

        def conv1_pe(ch, q):
            lo, hi = W1[q], W1[q + 1]
            n = hi - lo
            ps1 = psum.tile([128, QT], F32, tag="cps", name=f"c1ps_{ch}_{q}")
            for k in range(3):
                off = PAD - 1 + k + lo
                nc.tensor.matmul(ps1[:, 0:n], d1sb[:, k, ch, :],
                                 xB[:, ch, off:off + n],
                                 start=(k == 0), stop=(k == 2))
            nc.scalar.activation(rsl(ch, PAD + lo, PAD + hi), ps1[:, 0:n],
                                 AF.Relu, bias=b1sb[:, ch:ch + 1])

        def conv1_lastcol(ch):
            # final column t=2047 for PE channels (tiny DVE ops)
            sa = rsl(ch, PAD + 2047, PAD + 2048)
            xs = lambda d: xB[:, ch, PAD + 2047 + d:PAD + 2048 + d]
            nc.vector.tensor_scalar(sa, xs(-1), w1sb[:, ch, 0:1], None,
                                    OP.mult)
            nc.vector.scalar_tensor_tensor(sa, xs(0), w1sb[:, ch, 1:2], sa,
                                           OP.mult, OP.add)
            nc.vector.scalar_tensor_tensor(sa, xs(1), w1sb[:, ch, 2:3], sa,
                                           OP.mult, OP.add)
            nc.vector.tensor_scalar(sa, sa, b1sb[:, ch:ch + 1], 0.0,
                                    OP.add, OP.max)

        def conv1_chain(eng, ch, q):
            lo = W1[q]
            hi = 2048 if q == 3 else W1[q + 1]
            a = rsl(ch, PAD + lo, PAD + hi)
            xs = lambda d: xB[:, ch, PAD + lo + d:PAD + hi + d]
            eng.tensor_scalar(a, xs(-1), w1sb[:, ch, 0:1], None,
                              OP.mult)
            eng.scalar_tensor_tensor(a, xs(0), w1sb[:, ch, 1:2], a,
                                     OP.mult, OP.add)
            eng.scalar_tensor_tensor(a, xs(1), w1sb[:, ch, 2:3], a,
                                     OP.mult, OP.add)
            eng.tensor_scalar(a, a, b1sb[:, ch:ch + 1], 0.0,
                              OP.add, OP.max)

        def conv23_kouter(ch, qs):
            # k-outer over a quarter pair: each LDWEIGHTS serves len(qs)
            # back-to-back matmuls
            pss = {}
            for q in qs:
                pss[q] = psum.tile([128, QT], F32, tag="cps",
                                   name=f"c23ps_{ch}_{q}")
            for k in range(7):
                for q in qs:
                    lo = W23[q]
                    off = PAD - 3 + k + lo
                    nc.tensor.matmul(pss[q][:], d23sb[:, k, ch, :],
                                     rsl(ch, off, off + QT),
                                     start=(k == 0), stop=(k == 6))
            # edge-fix: the padded 7-tap composite differs from the true
            # conv3(conv2(.)) at the two outermost columns; patch the psum
            # with tiny DVE MACs before evacuation
            if 0 in qs:
                e = pss[0][:, 0:1]
                nc.vector.scalar_tensor_tensor(e, rsl(ch, PAD, PAD + 1),
                                               ecsb[:, ch, 0:1], e,
                                               OP.mult, OP.add)
                nc.vector.scalar_tensor_tensor(e, rsl(ch, PAD + 1, PAD + 2),
                                               ecsb[:, ch, 1:2], e,
                                               OP.mult, OP.add)
            if 3 in qs:
                e = pss[3][:, QT - 1:QT]
                nc.vector.scalar_tensor_tensor(e, rsl(ch, PAD + T - 2,
                                                      PAD + T - 1),
                                               ecsb[:, ch, 2:3], e,
                                               OP.mult, OP.add)
                nc.vector.scalar_tensor_tensor(e, rsl(ch, PAD + T - 1,
                                                      PAD + T),
                                               ecsb[:, ch, 3:4], e,
                                               OP.mult, OP.add)
            for q in qs:
                lo = W23[q]
                hh = q // 2
                col = ch * 1024 + lo - 1024 * hh
                nc.scalar.activation(c3h[hh][:, col:col + QT],
                                     pss[q][:], AF.Copy,
                                     accum_out=pools[:, ch, q:q + 1])

        def conv1_any(ch, q):
            if CH_ENG[ch] == 'P':
                conv1_pe(ch, q)
            else:
                conv1_chain(nc.vector, ch, q)

        # PE stream: conv1 q0..q2, conv23 q01 (fills the wait for scatter
        # g3), conv1 q3, conv23 q23 (with B->A transpose-back per channel)
        for ch in range(NCH):
            conv1_any(ch, 0)
        for ch in range(NCH):
            conv1_any(ch, 1)
        for ch in range(NCH):
            conv1_any(ch, 2)
        def back_xpose(h):
            # one xbar transpose for all channels of a half: DMA_TRANSPOSE
            # has ~2us fixed cost, so 2 big ones beat 16 small ones
            nc.sync.dma_start(out=sba(h), in_=c3h[h][:], transpose=True)

        for ch in range(NCH):
            conv23_kouter(ch, (0, 1))
        back_xpose(0)
        for ch in range(NCH):
            conv1_any(ch, 3)
            if CH_ENG[ch] == 'P':
                conv1_lastcol(ch)
        for ch in range(NCH):
            conv23_kouter(ch, (2, 3))
        back_xpose(1)

        # ---- SE MLP ----
        se_ps = ctx.enter_context(tc.tile_pool(name="seps", bufs=1,
                                               space="PSUM"))
        h_ps = se_ps.tile([H, 4], F32, tag="hps")
        for ch in range(NCH):
            nc.tensor.matmul(h_ps[:], fc1sb[:, ch, :], pools[:, ch, :],
                             start=(ch == 0), stop=(ch == NCH - 1))
        h_half = pool.tile([H, 4], F32, tag="h_half")
        nc.scalar.activation(h_half[:], h_ps[:], AF.Relu)
        h_sb = pool.tile([H, 1], F32, tag="hsb")
        nc.vector.tensor_reduce(h_sb[:], h_half[:], mybir.AxisListType.X,
                                OP.add)
        # replicate h across 128 cols (per-partition scalar mult of a ones
        # tile), then one matmul pair computes sigmoid(fc2^T h) broadcast
        # over all partitions
        nc.vector.tensor_scalar_mul(h_rep[:], ones_h[:], h_sb[:])
        g_ps = se_ps.tile([128, 1024], F32, tag="gps")
        nc.tensor.matmul(g_ps[:, 0:512], h_rep[:], fc2sb[:, 0:512],
                         start=True, stop=True)
        nc.tensor.matmul(g_ps[:, 512:1024], h_rep[:], fc2sb[:, 512:1024],
                         start=True, stop=True)
        nc.scalar.activation(gateA[:], g_ps[:], AF.Sigmoid)
        gateA3 = gateA[:].rearrange("p (ch c) -> p ch c", ch=NCH)

        # ---- output: gate mult + residual add + store, per 2-tile chunk;
        # work split DVE/Pool, bf16 store on scalar/sync HWDGE queues ----
        for q in range(8):
            hf, to = q // 4, (2 * q) % 8
            eng = nc.gpsimd if TAIL_ENG[q] == 'G' else nc.vector
            for t in (2 * q, 2 * q + 1):
                st = sba(hf)[:, :, t % 8, :]
                eng.tensor_tensor(st, st, gateA3, op=OP.mult)
            zt = zX[:, 2 * q:2 * q + 2, :].rearrange(
                "p th (ch c) -> p ch th c", ch=NCH)
            eng.tensor_tensor(zt, zt, sba(hf)[:, :, to:to + 2, :],
                              op=OP.add)
            dq = nc.scalar if q % 2 == 0 else nc.sync
            dq.dma_start(out_dst[:, 2 * q:2 * q + 2, :],
                         zX[:, 2 * q:2 * q + 2, :])

    nc.compile()
    return nc


def _prep_weights(ln_w, ln_b, w1, w2, w3, fc1, fc2):
    import ml_dtypes
    w1 = w1[:, 0, :].astype(np.float64)   # [C, 3]
    w2 = w2[:, 0, :].astype(np.float64)   # [C, 5]
    w3 = w3[:, 0, :].astype(np.float64)   # [C, 3]
    ln_w = ln_w.astype(np.float64)
    ln_b = ln_b.astype(np.float64)
    w1f = w1 * ln_w[:, None]
    b1 = (ln_b * w1.sum(axis=1))

    def to_plh(a):  # [C, K] -> [128, NCH, K]
        return np.ascontiguousarray(
            a.reshape(NCH, 128, -1).transpose(1, 0, 2)).astype(np.float32)

    w1p = to_plh(w1f)
    b1p = np.ascontiguousarray(b1.reshape(NCH, 128).T).astype(np.float32)
    fc1p = to_plh((fc1.astype(np.float64) / T).T)
    fc2p = np.ascontiguousarray(
        fc2.astype(np.float64).T.reshape(H, NCH * 128)).astype(
            ml_dtypes.bfloat16)

    w23 = np.stack([np.convolve(w3[c], w2[c]) for c in range(C)])  # [C, 7]
    # edge-fix coefficients (negated: they accumulate into the psum)
    ec = np.stack([-w3[:, 0] * w2[:, 3], -w3[:, 0] * w2[:, 4],
                   -w3[:, 2] * w2[:, 0], -w3[:, 2] * w2[:, 1]], axis=1)  # [C,4]
    w23p = to_plh(w23)
    ecp = to_plh(ec)

    def diags(wk):  # [C, K] -> [128, K, NCH, 128] bf16 (partition-major)
        K = wk.shape[1]
        d = np.zeros((K, NCH, 128, 128), np.float32)
        for k in range(K):
            for chh in range(NCH):
                np.fill_diagonal(d[k, chh], wk[chh * 128:(chh + 1) * 128, k])
        return np.ascontiguousarray(
            d.transpose(2, 0, 1, 3)).astype(ml_dtypes.bfloat16)

    return {"w1p": w1p, "b1p": b1p, "w23p": w23p, "ecp": ecp,
            "fc1p": fc1p, "fc2p": fc2p,
            "d1": diags(w1f), "d23": diags(w23)}


def kernel(x, residual_input, ln_w, ln_b, w1, w2, w3, fc1, fc2):
    x = np.asarray(x, dtype=np.float32)
    residual_input = np.asarray(residual_input, dtype=np.float32)
    wts = _prep_weights(np.asarray(ln_w), np.asarray(ln_b),
                        np.asarray(w1), np.asarray(w2), np.asarray(w3),
                        np.asarray(fc1), np.asarray(fc2))

    if "nc" not in _CACHE:
        _CACHE["nc"] = _build()
    nc = _CACHE["nc"]

    in_maps = []
    for b in range(B):
        m = {"x": np.ascontiguousarray(x[b]),
             "res": np.ascontiguousarray(residual_input[b])}
        m.update(wts)
        in_maps.append(m)
    res = run_bass_kernel_spmd(nc, in_maps, core_ids=list(range(N_CORES)))
    out = np.stack([res.results[i]["out"] for i in range(N_CORES)], axis=0)
    return out.astype(np.float32)
